# revision 1
# baseline (speedup 1.0000x reference)
"""MoE gate (nn_Gate) Trainium2 kernel.

Computes, for x[32768, 4096] f32, weight[8, 4096] f32, bias[8] f32:
    logits  = x @ weight.T
    scores  = sqrt(softplus(logits))
    indices = top2(scores + bias)
    weights = normalize(scores at indices)
returning (weights[32768, 2] f32, indices[32768, 2] int32).

Strategy (8 NeuronCores, data-parallel over tokens, no collectives):
  * Each core gets a [4096 tokens, 4096] shard. On host we transpose the
    shard to x^T [4096 D, 4096 T] and split into an fp16 hi/lo pair
    (hi = fp16(x), lo = fp16(x - hi)), which represents f32 to ~2^-24 and
    keeps DMA bytes identical to f32 (2 x 2B). fp16 matmuls run at full
    PE rate (1 cycle/row), so logits = hi@Whi + hi@Wlo + lo@Whi is
    f32-grade at 3 bf16-speed matmuls.
  * W^T (tiny) is the stationary operand; x^T streams 512 tokens/matmul.
    The three products per (d-chunk, token-block) go to three different
    PE column groups (tile_position), which both runs them concurrently
    and splits the PSUM accumulation chains (less f32 rounding noise).
  * logits^T partials are PE-transposed back to token-major, summed, and
    scored on-chip: softplus via range-reduced polynomial exp + ln1p
    (ACT LUT tables on this build lack Softplus and their Exp/Ln are only
    ~1e-5 accurate; polynomial evaluation keeps the biased-score error
    ~1e-7 so top-2 ordering matches an f32 reference), sqrt via ACT LUT
    + one Newton step, top-2 via DVE max8/max_index.
"""

import os
from contextlib import ExitStack

import numpy as np

T_FULL = 32768
D = 4096
E = 8
NCORES = 8
TPC = T_FULL // NCORES      # tokens per core
P = 128                     # partitions
DCH = D // P                # 32 contraction chunks
TB = 8                      # PSUM token banks
NT = TPC // TB              # 512 tokens per bank
G = TPC // P                # 32 token groups of 128
TOPK = 2
ROUTE_SCALE = 1.0

# exp(-x) on [-0.76, 0.76], Chebyshev-node fit, rel err ~1.8e-9
EXP_C = [
    0.9999999999999999, -0.9999999890886784, 0.49999999891101055,
    -0.1666669184450777, 0.04166669179667306, -0.008331765742365889,
    0.0013887323999906955, -0.00020202238804072677, 2.5162082342160214e-05,
]
# H(v) = ln((1+z)/(1-z))/z, v = z^2 in [0, 1/9], rel err ~1e-10
LN_C = [
    1.9999999998089943, 0.6666667902706496, 0.3999871119480547,
    0.28620208897656446, 0.21398543327861763, 0.2439397667369125,
]
LN2_HI = 0.693359375                     # 12-bit, m*LN2_HI exact in f32
LN2_LO = float(np.log(2.0) - 0.693359375)
NEG_INV_LN2 = -1.4426950408889634

_CACHE = {}


def _build_nc():
    import concourse.bacc as bacc
    import concourse.tile as tile
    import concourse.mybir as mybir

    F32 = mybir.dt.float32
    F16 = mybir.dt.float16
    I32 = mybir.dt.int32
    U32 = mybir.dt.uint32
    AF = mybir.ActivationFunctionType
    OP = mybir.AluOpType
    AX = mybir.AxisListType.X

    nc = bacc.Bacc("TRN2", target_bir_lowering=False, debug=False)

    xhi_d = nc.dram_tensor("xhi", [D, TPC], F16, kind="ExternalInput").ap()
    xlo_d = nc.dram_tensor("xlo", [D, TPC], F16, kind="ExternalInput").ap()
    whi_d = nc.dram_tensor("whi", [P, DCH, E], F16, kind="ExternalInput").ap()
    wlo_d = nc.dram_tensor("wlo", [P, DCH, E], F16, kind="ExternalInput").ap()
    br_d = nc.dram_tensor("bias_rep", [P, E], F32, kind="ExternalInput").ap()
    sel_d = nc.dram_tensor("sel", [104, E], F32, kind="ExternalInput").ap()
    wout_d = nc.dram_tensor("w_out", [P, G, TOPK], F32, kind="ExternalOutput").ap()
    iout_d = nc.dram_tensor("i_out", [P, G, TOPK], I32, kind="ExternalOutput").ap()

    with tile.TileContext(nc) as tc, ExitStack() as ctx:
        singles = ctx.enter_context(tc.tile_pool(name="singles", bufs=1))
        xpool = ctx.enter_context(tc.tile_pool(name="xpool", bufs=4))
        pspool = ctx.enter_context(tc.tile_pool(name="ps", bufs=8, space="PSUM"))
        lsbp = ctx.enter_context(tc.tile_pool(name="lsbp", bufs=2))
        ep = ctx.enter_context(tc.tile_pool(name="ep", bufs=1))
        sc = ctx.enter_context(tc.tile_pool(name="sc", bufs=2))

        whi = singles.tile([P, DCH, E], F16)
        nc.sync.dma_start(whi, whi_d)
        wlo = singles.tile([P, DCH, E], F16)
        nc.sync.dma_start(wlo, wlo_d)
        brep = singles.tile([P, E], F32)
        nc.sync.dma_start(brep, br_d)
        sel = singles.tile([104, E], F32)
        nc.sync.dma_start(sel, sel_d)

        accs = [pspool.tile([P, NT], F32, tag="ps", name=f"acc{i}")
                for i in range(TB)]

        # Zero the PSUM rows between the four partial-sum blocks: the
        # selection matmul contracts over rows 0:104 and uninitialized PSUM
        # could hold NaN; the accumulation target rows are overwritten by
        # start=True matmuls and must not be touched.
        for i in range(TB):
            nc.vector.memset(accs[i], 0.0)

        # ---- gate matmul: 3 products x 32 d-chunks x 8 token banks ----
        for d in range(DCH):
            xh = xpool.tile([P, TPC], F16, tag="xh")
            nc.sync.dma_start(xh, xhi_d[d * P:(d + 1) * P, :])
            xl = xpool.tile([P, TPC], F16, tag="xl")
            nc.scalar.dma_start(xl, xlo_d[d * P:(d + 1) * P, :])
            gm = 32 * (d // 16)  # main product: col group 0 for d<16, 1 for d>=16
            if os.environ.get("KBUILD_PHASE") == "dma":
                nc.vector.tensor_copy(accs[0][0:1, 0:1], xh[0:1, 0:1])
                nc.vector.tensor_copy(accs[0][0:1, 1:2], xl[0:1, 0:1])
                continue
            for tb in range(TB):
                rh = xh[:, tb * NT:(tb + 1) * NT]
                rl = xl[:, tb * NT:(tb + 1) * NT]
                acc = accs[tb]
                nc.tensor.matmul(
                    acc[gm:gm + E, :], whi[:, d, :], rh,
                    start=(d % 16 == 0), stop=(d % 16 == 15),
                    tile_position=(0, gm))
                nc.tensor.matmul(
                    acc[64:64 + E, :], wlo[:, d, :], rh,
                    start=(d == 0), stop=(d == DCH - 1),
                    tile_position=(0, 64))
                nc.tensor.matmul(
                    acc[96:96 + E, :], whi[:, d, :], rl,
                    start=(d == 0), stop=(d == DCH - 1),
                    tile_position=(0, 96))

        if os.environ.get("KBUILD_PHASE") in ("mm", "dma"):
            dummy = ep.tile([P, G, TOPK], F32, name="dummy")
            nc.vector.memset(dummy, 0.0)
            nc.vector.memset(ep.tile([P, G, TOPK], I32, name="dummy2"), 0)
            nc.sync.dma_start(wout_d, dummy)
            nc.compile()
            return nc

        # ---- transpose+combine via selection matmul, then score per half ----
        # sel[104, 8]: rows {e, 32+e, 64+e, 96+e} -> col e, so
        # lsb_slice.T @ sel = token-major logits with the 4 partials summed.
        ltok = ep.tile([P, G, E], F32)
        for tb in range(TB):
            lsb = lsbp.tile([104, NT], F32, tag="lsb", name=f"lsb{tb}")
            nc.scalar.activation(lsb, accs[tb][0:104, :], AF.Copy)
            for q in range(4):
                g = tb * 4 + q
                pt = pspool.tile([P, E], F32, tag="ps", name=f"pt{g}")
                nc.tensor.matmul(pt, lsb[:, q * P:(q + 1) * P], sel,
                                 start=True, stop=True)
                nc.vector.tensor_copy(ltok[:, g, :], pt)

        # ---- scoring + top2 + normalize, in two g-halves for overlap ----
        maxb = ep.tile([P, G, E], F32)
        idxb = ep.tile([P, G, E], U32)
        wpair = ep.tile([P, G, TOPK], F32)
        wout = ep.tile([P, G, TOPK], F32)
        iout = ep.tile([P, G, TOPK], I32)

        def score_slice(g0, g1):
            gs = g1 - g0
            sh = [P, gs, E]

            def f32t(name):
                return sc.tile(sh, F32, tag=name, name=f"{name}_{g0}")

            L = ltok[:, g0:g1, :]
            a = f32t("a")
            nc.vector.tensor_scalar(a[:].bitcast(I32), L.bitcast(I32),
                                    0x7FFFFFFF, None, op0=OP.bitwise_and)
            yn = f32t("yn")
            nc.vector.tensor_scalar_mul(yn, a, NEG_INV_LN2)
            mi = sc.tile(sh, I32, tag="mi", name=f"mi_{g0}")
            nc.vector.tensor_copy(mi, yn)                  # f32 -> i32
            mf = f32t("mf")
            nc.vector.tensor_copy(mf, mi)                  # i32 -> f32
            g2 = f32t("g2")
            nc.vector.scalar_tensor_tensor(g2, mf, LN2_HI, a, op0=OP.mult, op1=OP.add)
            nc.vector.scalar_tensor_tensor(g2, mf, LN2_LO, g2, op0=OP.mult, op1=OP.add)
            rt = f32t("rt")
            deg = len(EXP_C) - 1
            nc.vector.tensor_scalar_mul(rt, g2, EXP_C[deg])
            for k in range(deg - 1, 0, -1):
                nc.vector.scalar_tensor_tensor(rt, rt, EXP_C[k], g2, op0=OP.add, op1=OP.mult)
            p = f32t("p")
            nc.vector.tensor_scalar_add(p, rt, EXP_C[0])
            eb = sc.tile(sh, I32, tag="eb", name=f"eb_{g0}")
            nc.vector.tensor_scalar_add(eb, mi, 127)
            nc.vector.tensor_scalar(eb, eb, 23, None, op0=OP.logical_shift_left)
            t = f32t("t")
            nc.vector.tensor_mul(t, p, eb[:].bitcast(F32))
            den = f32t("den")
            nc.vector.tensor_scalar_add(den, t, 2.0)
            rd = f32t("rd")
            nc.vector.reciprocal(rd, den)
            m0 = f32t("m0")
            nc.vector.tensor_mul(m0, den, rd)
            nc.vector.tensor_scalar_mul(m0, m0, -1.0)
            nc.vector.scalar_tensor_tensor(rd, m0, 2.0, rd, op0=OP.add, op1=OP.mult)
            z = f32t("z")
            nc.vector.tensor_mul(z, t, rd)
            v = f32t("v")
            nc.vector.tensor_mul(v, z, z)
            ldeg = len(LN_C) - 1
            nc.vector.tensor_scalar_mul(rt, v, LN_C[ldeg])
            for k in range(ldeg - 1, 0, -1):
                nc.vector.scalar_tensor_tensor(rt, rt, LN_C[k], v, op0=OP.add, op1=OP.mult)
            hq = f32t("hq")
            nc.vector.tensor_scalar_add(hq, rt, LN_C[0])
            u = f32t("u")
            nc.vector.tensor_mul(u, z, hq)
            sp = f32t("sp")
            nc.vector.tensor_scalar_max(sp, L, 0.0)
            nc.vector.tensor_add(sp, sp, u)
            s0 = f32t("s0")
            nc.scalar.activation(s0, sp, AF.Sqrt)
            rs = f32t("rs")
            nc.vector.reciprocal(rs, s0)
            m1 = f32t("m1")
            nc.vector.tensor_mul(m1, s0, rs)
            nc.vector.tensor_scalar_mul(m1, m1, -1.0)
            nc.vector.scalar_tensor_tensor(rs, m1, 2.0, rs, op0=OP.add, op1=OP.mult)
            s = f32t("s")
            nc.vector.tensor_mul(s, sp, rs)
            nc.vector.tensor_add(s, s, s0)
            nc.vector.tensor_scalar_mul(s, s, 0.5)
            biased = f32t("biased")
            brep_b = brep[:].unsqueeze(1).broadcast_to(sh)
            nc.vector.tensor_add(biased, s, brep_b)

            for g in range(g0, g1):
                gl = g - g0
                nc.vector.max(maxb[:, g, :], biased[:, gl, :])
                nc.vector.max_index(idxb[:, g, :], maxb[:, g, :], biased[:, gl, :])
            oh = f32t("oh")
            tt = f32t("tt")
            for j in range(TOPK):
                mj = maxb[:, g0:g1, j:j + 1].broadcast_to(sh)
                nc.vector.tensor_tensor(oh, biased, mj, op=OP.is_equal)
                nc.vector.tensor_mul(tt, oh, s)
                nc.vector.reduce_max(wpair[:, g0:g1, j], tt, axis=AX)
            ssum = sc.tile([P, gs], F32, tag="ssum", name=f"ssum_{g0}")
            nc.vector.reduce_sum(ssum, wpair[:, g0:g1, :], axis=AX)
            r0 = sc.tile([P, gs], F32, tag="r0", name=f"r0_{g0}")
            nc.vector.reciprocal(r0, ssum)
            m2 = sc.tile([P, gs], F32, tag="m2", name=f"m2_{g0}")
            nc.vector.tensor_mul(m2, ssum, r0)
            nc.vector.tensor_scalar_mul(m2, m2, -1.0)
            nc.vector.scalar_tensor_tensor(r0, m2, 2.0, r0, op0=OP.add, op1=OP.mult)
            r0b = r0[:].unsqueeze(2).broadcast_to([P, gs, TOPK])
            nc.vector.tensor_tensor(wout[:, g0:g1, :], wpair[:, g0:g1, :], r0b,
                                    op=OP.mult)
            nc.vector.tensor_copy(iout[:, g0:g1, :],
                                  idxb[:, g0:g1, 0:TOPK].bitcast(I32))

        score_slice(0, G // 2)
        score_slice(G // 2, G)
        nc.sync.dma_start(wout_d, wout)
        nc.sync.dma_start(iout_d, iout)

    nc.compile()
    return nc


def _prep_inputs(x, weight, bias):
    f16 = np.float16
    wt = np.ascontiguousarray(weight.T).astype(np.float32)      # [D, E]
    whi = wt.astype(f16)
    wlo = (wt - whi.astype(np.float32)).astype(f16)
    # reorder [D, E] -> [P, DCH, E] so the SBUF load is one contiguous DMA
    whi_sb = np.ascontiguousarray(whi.reshape(DCH, P, E).transpose(1, 0, 2))
    wlo_sb = np.ascontiguousarray(wlo.reshape(DCH, P, E).transpose(1, 0, 2))
    brep = np.ascontiguousarray(np.broadcast_to(bias.astype(np.float32), (P, E)))
    sel = np.zeros((104, E), np.float32)
    for e in range(E):
        for blk in range(4):
            sel[32 * blk + e, e] = 1.0

    in_maps = []
    for c in range(NCORES):
        xs = x[c * TPC:(c + 1) * TPC]
        xT = np.ascontiguousarray(xs.T).astype(np.float32)      # [D, TPC]
        xhi = xT.astype(f16)
        xlo = (xT - xhi.astype(np.float32)).astype(f16)
        in_maps.append({
            "xhi": xhi, "xlo": xlo,
            "whi": whi_sb, "wlo": wlo_sb,
            "bias_rep": brep, "sel": sel,
        })
    return in_maps


def kernel(x, weight, bias):
    x = np.asarray(x, dtype=np.float32)
    weight = np.asarray(weight, dtype=np.float32)
    bias = np.asarray(bias, dtype=np.float32)
    assert x.shape == (T_FULL, D) and weight.shape == (E, D) and bias.shape == (E,)

    from concourse.bass_utils import run_bass_kernel_spmd

    if "nc" not in _CACHE:
        _CACHE["nc"] = _build_nc()
    nc = _CACHE["nc"]

    in_maps = _prep_inputs(x, weight, bias)
    res = run_bass_kernel_spmd(nc, in_maps, core_ids=list(range(NCORES)),
                               trace=bool(os.environ.get("BASS_TRACE")))
    _CACHE["last_results"] = res

    weights = np.empty((T_FULL, TOPK), np.float32)
    indices = np.empty((T_FULL, TOPK), np.int32)
    for c in range(NCORES):
        w_c = res.results[c]["w_out"]                 # [P, G, 2], token = g*128+p
        i_c = res.results[c]["i_out"]
        weights[c * TPC:(c + 1) * TPC] = w_c.transpose(1, 0, 2).reshape(TPC, TOPK)
        indices[c * TPC:(c + 1) * TPC] = i_c.transpose(1, 0, 2).reshape(TPC, TOPK)
    if ROUTE_SCALE != 1.0:
        weights *= ROUTE_SCALE
    return weights, indices



# revision 2
# speedup vs baseline: 1.0444x; 1.0444x over previous
"""MoE gate (nn_Gate) Trainium2 kernel, v2.

Computes, for x[32768, 4096] f32, weight[8, 4096] f32, bias[8] f32:
    logits  = x @ weight.T
    scores  = sqrt(softplus(logits))
    indices = top2(scores + bias)
    weights = normalize(scores at indices)
returning (weights[32768, 2] f32, indices[32768, 2] int32).

Strategy (8 NeuronCores, data-parallel over tokens, no collectives):
  * 3 bytes/element streamed per x value instead of 4: xh = fp16(x) plus
    xl8 = fp8e3m4((x - xh) * 2^11). The fp8 lo plane carries ~5 extra
    mantissa bits, giving ~2^-16 relative logit accuracy -- enough to
    reproduce the f32 reference's top-2 ordering except on exact ties
    (empirically 1 token in 32768, same as a full hi/lo fp16 split).
    DMA drops from 64 MiB to 48 MiB per core.
  * One fp16 matmul per (chunk, block) with a packed [whi | wlo] 16-wide
    stationary computes both hi products from a single xh stream; the lo
    product xl8 @ w8 (both fp8e3m4) streams once more at fp8 rate.
  * Tokens are processed in 512-token blocks so that the transpose +
    sqrtsoftplus + top-2 + normalize of block b overlaps the DMA/matmul
    streaming of block b+1; only the last block's scoring is exposed.
  * PSUM rows per block-accumulator: 0:16 hi products, 32:40 lo product
    (rows 16:32 zeroed once); a [40, 8] f32 selection matmul transposes
    to token-major and folds in the 2^-16 lo-plane scale in one shot.
  * Scoring: softplus via range-reduced polynomial exp + ln1p (ACT LUT
    Exp/Ln on this build are only ~1e-5 accurate; polynomials keep the
    biased-score error ~1e-7 so top-2 ordering matches f32), sqrt via
    ACT LUT + one Newton step, top-2 via DVE max8/max_index.
"""

import os
from contextlib import ExitStack

import numpy as np

T_FULL = 32768
D = 4096
E = 8
NCORES = 8
TPC = T_FULL // NCORES      # tokens per core
P = 128                     # partitions
DCH = D // P                # 32 contraction chunks
S = 512                     # tokens per block (one PSUM bank)
NB = TPC // S               # 8 blocks
GPB = S // P                # 4 groups of 128 tokens per block
G = TPC // P                # 32 groups per core
KHI = 8                     # d-chunks per xh DMA
KLO = 16                    # d-chunks per xl DMA
TOPK = 2
ROUTE_SCALE = 1.0
XSCALE = 2.0 ** 11          # lo plane pre-scale
WSCALE = 2.0 ** 5           # lo-plane weight pre-scale
LO_COMB = 1.0 / (XSCALE * WSCALE)

# exp(-x) on [-0.76, 0.76], Chebyshev-node fit, rel err ~1.8e-9
EXP_C = [
    0.9999999999999999, -0.9999999890886784, 0.49999999891101055,
    -0.1666669184450777, 0.04166669179667306, -0.008331765742365889,
    0.0013887323999906955, -0.00020202238804072677, 2.5162082342160214e-05,
]
# H(v) = ln((1+z)/(1-z))/z, v = z^2 in [0, 1/9], rel err ~1e-10
LN_C = [
    1.9999999998089943, 0.6666667902706496, 0.3999871119480547,
    0.28620208897656446, 0.21398543327861763, 0.2439397667369125,
]
LN2_HI = 0.693359375                     # 12-bit, m*LN2_HI exact in f32
LN2_LO = float(np.log(2.0) - 0.693359375)
NEG_INV_LN2 = -1.4426950408889634

_CACHE = {}


def _build_nc():
    import concourse.bacc as bacc
    import concourse.tile as tile
    import concourse.mybir as mybir

    F32 = mybir.dt.float32
    F16 = mybir.dt.float16
    F8 = mybir.dt.float8e3
    I32 = mybir.dt.int32
    U32 = mybir.dt.uint32
    AF = mybir.ActivationFunctionType
    OP = mybir.AluOpType
    AX = mybir.AxisListType.X

    nc = bacc.Bacc("TRN2", target_bir_lowering=False, debug=False)

    xh_d = nc.dram_tensor("xh", [P, DCH, TPC], F16, kind="ExternalInput").ap()
    xl_d = nc.dram_tensor("xl8", [P, DCH, TPC], F8, kind="ExternalInput").ap()
    wp_d = nc.dram_tensor("wpair", [P, DCH, 2 * E], F16, kind="ExternalInput").ap()
    w8_d = nc.dram_tensor("w8", [P, DCH, E], F8, kind="ExternalInput").ap()
    br_d = nc.dram_tensor("bias_rep", [P, E], F32, kind="ExternalInput").ap()
    sel_d = nc.dram_tensor("sel", [40, E], F32, kind="ExternalInput").ap()
    wout_d = nc.dram_tensor("w_out", [P, G, TOPK], F32, kind="ExternalOutput").ap()
    iout_d = nc.dram_tensor("i_out", [P, G, TOPK], I32, kind="ExternalOutput").ap()

    with tile.TileContext(nc) as tc, ExitStack() as ctx:
        singles = ctx.enter_context(tc.tile_pool(name="singles", bufs=1))
        xhp = ctx.enter_context(tc.tile_pool(name="xhp", bufs=10))
        xlp = ctx.enter_context(tc.tile_pool(name="xlp", bufs=6))
        pspool = ctx.enter_context(tc.tile_pool(name="ps", bufs=3, space="PSUM"))
        ptpool = ctx.enter_context(tc.tile_pool(name="pt", bufs=2, space="PSUM"))
        lsbp = ctx.enter_context(tc.tile_pool(name="lsbp", bufs=2))
        ep = ctx.enter_context(tc.tile_pool(name="ep", bufs=1))
        sc = ctx.enter_context(tc.tile_pool(name="sc", bufs=2))

        wpair = singles.tile([P, DCH, 2 * E], F16)
        nc.sync.dma_start(wpair, wp_d)
        w8 = singles.tile([P, DCH, E], F8)
        nc.sync.dma_start(w8, w8_d)
        brep = singles.tile([P, E], F32)
        nc.sync.dma_start(brep, br_d)
        sel = singles.tile([40, E], F32)
        nc.sync.dma_start(sel, sel_d)

        accs = [pspool.tile([P, S], F32, tag="ps", name=f"acc{i}")
                for i in range(3)]
        # Rows 16:32 sit between the two matmul target groups and are read
        # by the selection matmul (sel rows there are 0); zero them once so
        # 0 * garbage can't produce NaN. Engine writes must start at a
        # 32-aligned partition, so zero 0:32 (0:16 is overwritten by the
        # first start=True matmul).
        for a in accs:
            nc.vector.memset(a[0:32, :], 0.0)

        # persistent outputs + scoring state
        ltok = ep.tile([P, G, E], F32)
        maxb = ep.tile([P, G, E], F32)
        idxb = ep.tile([P, G, E], U32)
        wpairs = ep.tile([P, G, TOPK], F32)
        wout = ep.tile([P, G, TOPK], F32)
        iout = ep.tile([P, G, TOPK], I32)

        def score_block(b):
            g0, g1 = b * GPB, (b + 1) * GPB
            gs = GPB
            sh = [P, gs, E]

            def f32t(name):
                return sc.tile(sh, F32, tag=name, name=f"{name}_{b}")

            L = ltok[:, g0:g1, :]
            a = f32t("a")
            nc.vector.tensor_scalar(a[:].bitcast(I32), L.bitcast(I32),
                                    0x7FFFFFFF, None, op0=OP.bitwise_and)
            yn = f32t("yn")
            nc.vector.tensor_scalar_mul(yn, a, NEG_INV_LN2)
            mi = sc.tile(sh, I32, tag="mi", name=f"mi_{b}")
            nc.vector.tensor_copy(mi, yn)                  # f32 -> i32
            mf = f32t("mf")
            nc.vector.tensor_copy(mf, mi)                  # i32 -> f32
            g2 = f32t("g2")
            nc.vector.scalar_tensor_tensor(g2, mf, LN2_HI, a, op0=OP.mult, op1=OP.add)
            nc.vector.scalar_tensor_tensor(g2, mf, LN2_LO, g2, op0=OP.mult, op1=OP.add)
            rt = f32t("rt")
            deg = len(EXP_C) - 1
            nc.vector.tensor_scalar_mul(rt, g2, EXP_C[deg])
            for k in range(deg - 1, 0, -1):
                nc.vector.scalar_tensor_tensor(rt, rt, EXP_C[k], g2, op0=OP.add, op1=OP.mult)
            p = f32t("p")
            nc.vector.tensor_scalar_add(p, rt, EXP_C[0])
            eb = sc.tile(sh, I32, tag="eb", name=f"eb_{b}")
            nc.vector.tensor_scalar_add(eb, mi, 127)
            nc.vector.tensor_scalar(eb, eb, 23, None, op0=OP.logical_shift_left)
            t = f32t("t")
            nc.vector.tensor_mul(t, p, eb[:].bitcast(F32))
            den = f32t("den")
            nc.vector.tensor_scalar_add(den, t, 2.0)
            rd = f32t("rd")
            nc.vector.reciprocal(rd, den)
            m0 = f32t("m0")
            nc.vector.tensor_mul(m0, den, rd)
            nc.vector.tensor_scalar_mul(m0, m0, -1.0)
            nc.vector.scalar_tensor_tensor(rd, m0, 2.0, rd, op0=OP.add, op1=OP.mult)
            z = f32t("z")
            nc.vector.tensor_mul(z, t, rd)
            v = f32t("v")
            nc.vector.tensor_mul(v, z, z)
            ldeg = len(LN_C) - 1
            nc.vector.tensor_scalar_mul(rt, v, LN_C[ldeg])
            for k in range(ldeg - 1, 0, -1):
                nc.vector.scalar_tensor_tensor(rt, rt, LN_C[k], v, op0=OP.add, op1=OP.mult)
            hq = f32t("hq")
            nc.vector.tensor_scalar_add(hq, rt, LN_C[0])
            u = f32t("u")
            nc.vector.tensor_mul(u, z, hq)
            sp = f32t("sp")
            nc.vector.tensor_scalar_max(sp, L, 0.0)
            nc.vector.tensor_add(sp, sp, u)
            s0 = f32t("s0")
            nc.scalar.activation(s0, sp, AF.Sqrt)
            rs = f32t("rs")
            nc.vector.reciprocal(rs, s0)
            m1 = f32t("m1")
            nc.vector.tensor_mul(m1, s0, rs)
            nc.vector.tensor_scalar_mul(m1, m1, -1.0)
            nc.vector.scalar_tensor_tensor(rs, m1, 2.0, rs, op0=OP.add, op1=OP.mult)
            s = f32t("s")
            nc.vector.tensor_mul(s, sp, rs)
            nc.vector.tensor_add(s, s, s0)
            nc.vector.tensor_scalar_mul(s, s, 0.5)
            biased = f32t("biased")
            brep_b = brep[:].unsqueeze(1).broadcast_to(sh)
            nc.vector.tensor_add(biased, s, brep_b)

            for g in range(g0, g1):
                gl = g - g0
                nc.vector.max(maxb[:, g, :], biased[:, gl, :])
                nc.vector.max_index(idxb[:, g, :], maxb[:, g, :], biased[:, gl, :])
            oh = f32t("oh")
            tt = f32t("tt")
            for j in range(TOPK):
                mj = maxb[:, g0:g1, j:j + 1].broadcast_to(sh)
                nc.vector.tensor_tensor(oh, biased, mj, op=OP.is_equal)
                nc.vector.tensor_mul(tt, oh, s)
                nc.vector.reduce_max(wpairs[:, g0:g1, j], tt, axis=AX)
            ssum = sc.tile([P, gs], F32, tag="ssum", name=f"ssum_{b}")
            nc.vector.reduce_sum(ssum, wpairs[:, g0:g1, :], axis=AX)
            r0 = sc.tile([P, gs], F32, tag="r0", name=f"r0_{b}")
            nc.vector.reciprocal(r0, ssum)
            m2 = sc.tile([P, gs], F32, tag="m2", name=f"m2_{b}")
            nc.vector.tensor_mul(m2, ssum, r0)
            nc.vector.tensor_scalar_mul(m2, m2, -1.0)
            nc.vector.scalar_tensor_tensor(r0, m2, 2.0, r0, op0=OP.add, op1=OP.mult)
            r0b = r0[:].unsqueeze(2).broadcast_to([P, gs, TOPK])
            nc.vector.tensor_tensor(wout[:, g0:g1, :], wpairs[:, g0:g1, :], r0b,
                                    op=OP.mult)
            nc.vector.tensor_copy(iout[:, g0:g1, :],
                                  idxb[:, g0:g1, 0:TOPK].bitcast(I32))
            nc.scalar.dma_start(wout_d[:, g0:g1, :], wout[:, g0:g1, :])
            nc.scalar.dma_start(iout_d[:, g0:g1, :], iout[:, g0:g1, :])

        for b in range(NB):
            t0, t1 = b * S, (b + 1) * S
            # issue in consumption order (PE walks d ascending): each lo
            # group right after the hi group it unblocks
            xh_g = [None] * (DCH // KHI)
            xl_g = [None] * (DCH // KLO)

            def load_hi(i):
                xht = xhp.tile([P, KHI, S], F16, tag="xh")
                nc.sync.dma_start(xht, xh_d[:, i * KHI:(i + 1) * KHI, t0:t1])
                xh_g[i] = xht

            def load_lo(j):
                xlt = xlp.tile([P, KLO, S], F8, tag="xl")
                nc.scalar.dma_start(xlt, xl_d[:, j * KLO:(j + 1) * KLO, t0:t1])
                xl_g[j] = xlt

            load_hi(0), load_lo(0), load_hi(1)
            load_hi(2), load_lo(1), load_hi(3)

            acc = accs[b % 3]
            for d in range(DCH):
                xh_t = xh_g[d // KHI]
                xl_t = xl_g[d // KLO]
                nc.tensor.matmul(
                    acc[0:2 * E, :], wpair[:, d, :], xh_t[:, d % KHI, :],
                    start=(d == 0), stop=(d == DCH - 1),
                    tile_position=(0, 0))
                nc.tensor.matmul(
                    acc[32:32 + E, :], w8[:, d, :], xl_t[:, d % KLO, :],
                    start=(d == 0), stop=(d == DCH - 1),
                    tile_position=(0, 32))

            # transpose+combine via selection matmul: token-major logits
            lsb = lsbp.tile([40, S], F32, tag="lsb", name=f"lsb{b % 2}")
            nc.scalar.activation(lsb, acc[0:40, :], AF.Copy)
            pt = ptpool.tile([P, GPB, E], F32, tag="pt", name=f"pt{b % 2}")
            for q in range(GPB):
                nc.tensor.matmul(pt[:, q, :], lsb[:, q * P:(q + 1) * P], sel,
                                 start=True, stop=True)
            nc.vector.tensor_copy(ltok[:, b * GPB:(b + 1) * GPB, :], pt)

            score_block(b)

    nc.compile()
    return nc


def _prep_inputs(x, weight, bias):
    import ml_dtypes
    f16 = np.float16
    F8 = ml_dtypes.float8_e3m4
    wt = np.ascontiguousarray(weight.T).astype(np.float32)      # [D, E]
    whi = wt.astype(f16)
    wlo = (wt - whi.astype(np.float32)).astype(f16)
    w8 = (wt * np.float32(WSCALE)).astype(F8)
    # [D, E] -> [P, DCH, E] so chunk d's stationary is wpair[:, d, :]
    def to_chunks(a):
        return np.ascontiguousarray(
            a.reshape(DCH, P, a.shape[-1]).transpose(1, 0, 2))
    wpair = np.concatenate([to_chunks(whi), to_chunks(wlo)], axis=2)
    w8_sb = to_chunks(w8)
    brep = np.ascontiguousarray(np.broadcast_to(bias.astype(np.float32), (P, E)))
    sel = np.zeros((40, E), np.float32)
    for e in range(E):
        sel[e, e] = 1.0
        sel[E + e, e] = 1.0
        sel[32 + e, e] = LO_COMB

    in_maps = []
    for c in range(NCORES):
        xs = x[c * TPC:(c + 1) * TPC]
        xT = np.ascontiguousarray(xs.T).astype(np.float32)      # [D, TPC]
        xhi = xT.astype(f16)
        xl8 = ((xT - xhi.astype(np.float32)) * np.float32(XSCALE)).astype(F8)
        xh_r = np.ascontiguousarray(
            xhi.reshape(DCH, P, TPC).transpose(1, 0, 2))        # [P, DCH, TPC]
        xl_r = np.ascontiguousarray(
            xl8.reshape(DCH, P, TPC).transpose(1, 0, 2))
        in_maps.append({
            "xh": xh_r, "xl8": xl_r,
            "wpair": wpair, "w8": w8_sb,
            "bias_rep": brep, "sel": sel,
        })
    return in_maps


def kernel(x, weight, bias):
    x = np.asarray(x, dtype=np.float32)
    weight = np.asarray(weight, dtype=np.float32)
    bias = np.asarray(bias, dtype=np.float32)
    assert x.shape == (T_FULL, D) and weight.shape == (E, D) and bias.shape == (E,)

    from concourse.bass_utils import run_bass_kernel_spmd

    if "nc" not in _CACHE:
        _CACHE["nc"] = _build_nc()
    nc = _CACHE["nc"]

    in_maps = _prep_inputs(x, weight, bias)
    res = run_bass_kernel_spmd(nc, in_maps, core_ids=list(range(NCORES)),
                               trace=bool(os.environ.get("BASS_TRACE")))
    _CACHE["last_results"] = res

    weights = np.empty((T_FULL, TOPK), np.float32)
    indices = np.empty((T_FULL, TOPK), np.int32)
    for c in range(NCORES):
        w_c = res.results[c]["w_out"]                 # [P, G, 2], token = g*128+p
        i_c = res.results[c]["i_out"]
        weights[c * TPC:(c + 1) * TPC] = w_c.transpose(1, 0, 2).reshape(TPC, TOPK)
        indices[c * TPC:(c + 1) * TPC] = i_c.transpose(1, 0, 2).reshape(TPC, TOPK)
    if ROUTE_SCALE != 1.0:
        weights *= ROUTE_SCALE
    return weights, indices


# revision 3
# speedup vs baseline: 1.0528x; 1.0080x over previous
"""MoE gate (nn_Gate) Trainium2 kernel.

Computes, for x[32768, 4096] f32, weight[8, 4096] f32, bias[8] f32:
    logits  = x @ weight.T
    scores  = sqrt(softplus(logits))
    indices = top2(scores + bias)
    weights = normalize(scores at indices)
returning (weights[32768, 2] f32, indices[32768, 2] int32).

Strategy (8 NeuronCores, data-parallel over tokens, no collectives):
  * 3 bytes/element streamed per x value instead of 4: xh = fp16(x) plus
    xl8 = fp8e3m4((x - xh) * 2^11). The fp8 lo plane carries ~5 extra
    mantissa bits, giving ~2^-16 relative logit accuracy -- enough to
    reproduce the f32 reference's top-2 ordering except on exact ties
    (empirically 1 token in 32768, same as a full hi/lo fp16 split).
    DMA drops from 64 MiB to 48 MiB per core; the DMA engines (360 GB/s
    shared across queues) are the roofline at ~140 us.
  * One fp16 matmul per (chunk, block) with a packed [whi | wlo] 16-wide
    stationary computes both hi products from a single xh stream; the lo
    product xl8 @ w8 (both fp8e3m4) streams once more at fp8 rate.
  * 512-token blocks; block b's transpose + scoring hides under block
    b+1's streaming. Queue discipline keeps the stream dense: sync (SP)
    carries only xh DMAs, scalar (Activation) only xl DMAs + the sqrt;
    the PSUM->SBUF logit copy runs on DVE, and output DMAs are deferred
    one block so their semaphore waits never stall a stream queue.
  * PSUM rows per block-accumulator: 0:16 hi products, 32:40 lo product
    (rows 16:32 zeroed once); a [40, 8] f32 selection matmul transposes
    to token-major and folds in the 2^-16 lo-plane scale in one shot.
    Scoring reads logits straight from the PSUM transpose tile.
  * Scoring: softplus via range-reduced polynomial exp + ln1p (ACT LUT
    Exp/Ln on this build are only ~1e-5 accurate; polynomials keep the
    biased-score error ~1e-7 so top-2 ordering matches f32), sqrt via
    ACT LUT + one Newton step, top-2 via DVE max8/max_index. The three
    reciprocals use the custom-DVE approx ops (22-bit for the two that
    feed the ordering, 18-bit for the final normalize) instead of
    explicit Newton chains.
"""

import os
from contextlib import ExitStack

import numpy as np

T_FULL = 32768
D = 4096
E = 8
NCORES = 8
TPC = T_FULL // NCORES      # tokens per core
P = 128                     # partitions
DCH = D // P                # 32 contraction chunks
S = 512                     # tokens per block (one PSUM bank)
NB = TPC // S               # 8 blocks
GPB = S // P                # 4 groups of 128 tokens per block
G = TPC // P                # 32 groups per core
KHI = 8                     # d-chunks per xh DMA
KLO = 16                    # d-chunks per xl DMA
TOPK = 2
ROUTE_SCALE = 1.0
XSCALE = 2.0 ** 11          # lo plane pre-scale
WSCALE = 2.0 ** 5           # lo-plane weight pre-scale
LO_COMB = 1.0 / (XSCALE * WSCALE)

# exp(-x) on [-0.36, 0.71] (covers either f32->i32 rounding semantics of
# the range reduction), Chebyshev fit, rel err ~6.5e-8
EXP_C = [
    1.0000000248079328, -0.9999996937194584, 0.4999974889120239,
    -0.1666744435603411, 0.04171073643046914, -0.008316109711575575,
    0.0011778843053845457,
]
# H(v) = ln((1+z)/(1-z))/z, v = z^2 in [0, 1/9], rel err ~1e-10
LN_C = [
    1.9999999998089943, 0.6666667902706496, 0.3999871119480547,
    0.28620208897656446, 0.21398543327861763, 0.2439397667369125,
]
LN2_HI = 0.693359375                     # 12-bit, m*LN2_HI exact in f32
LN2_LO = float(np.log(2.0) - 0.693359375)
NEG_INV_LN2 = -1.4426950408889634

_CACHE = {}


def _build_nc():
    import concourse.bacc as bacc
    import concourse.tile as tile
    import concourse.mybir as mybir

    F32 = mybir.dt.float32
    F16 = mybir.dt.float16
    F8 = mybir.dt.float8e4
    I32 = mybir.dt.int32
    U32 = mybir.dt.uint32
    AF = mybir.ActivationFunctionType
    OP = mybir.AluOpType
    AX = mybir.AxisListType.X

    nc = bacc.Bacc("TRN2", target_bir_lowering=False, debug=False)

    xh_d = nc.dram_tensor("xh", [P, DCH, TPC], F16, kind="ExternalInput").ap()
    xl_d = nc.dram_tensor("xl8", [P, DCH, TPC], F8, kind="ExternalInput").ap()
    wp_d = nc.dram_tensor("wpair", [P, DCH, 2 * E], F16, kind="ExternalInput").ap()
    w8_d = nc.dram_tensor("w8", [P, DCH, 2 * E], F8, kind="ExternalInput").ap()
    br_d = nc.dram_tensor("bias_rep", [P, E], F32, kind="ExternalInput").ap()
    sel_d = nc.dram_tensor("sel", [48, E], F32, kind="ExternalInput").ap()
    wout_d = nc.dram_tensor("w_out", [P, G, TOPK], F32, kind="ExternalOutput").ap()
    iout_d = nc.dram_tensor("i_out", [P, G, TOPK], I32, kind="ExternalOutput").ap()

    with tile.TileContext(nc) as tc, ExitStack() as ctx:
        singles = ctx.enter_context(tc.tile_pool(name="singles", bufs=1))
        xhp = ctx.enter_context(tc.tile_pool(name="xhp", bufs=10))
        xlp = ctx.enter_context(tc.tile_pool(name="xlp", bufs=6))
        pspool = ctx.enter_context(tc.tile_pool(name="ps", bufs=3, space="PSUM"))
        ptpool = ctx.enter_context(tc.tile_pool(name="pt", bufs=3, space="PSUM"))
        lsbp = ctx.enter_context(tc.tile_pool(name="lsbp", bufs=2))
        ep = ctx.enter_context(tc.tile_pool(name="ep", bufs=1))
        sc = ctx.enter_context(tc.tile_pool(name="sc", bufs=2))

        wpair = singles.tile([P, DCH, 2 * E], F16)
        nc.sync.dma_start(wpair, wp_d)
        w8 = singles.tile([P, DCH, 2 * E], F8)
        nc.scalar.dma_start(w8, w8_d)
        # brep/sel are needed only ~20us in (first transpose/score); issue
        # them behind block 0's first lo DMA so they don't delay the stream
        brep = singles.tile([P, E], F32)
        sel = singles.tile([48, E], F32)
        deferred = [lambda: nc.scalar.dma_start(brep, br_d),
                    lambda: nc.scalar.dma_start(sel, sel_d)]

        accs = [pspool.tile([P, S], F32, tag="ps", name=f"acc{i}")
                for i in range(3)]
        # Rows 16:32 sit between the two matmul target groups and are read
        # by the selection matmul (sel rows there are 0); zero them once so
        # 0 * garbage can't produce NaN. Engine writes must start at a
        # 32-aligned partition, so zero 0:32 (0:16 is overwritten by the
        # first start=True matmul).
        for a in accs:
            nc.vector.memset(a[0:32, :], 0.0)

        # persistent outputs + scoring state
        maxb = ep.tile([P, G, E], F32)
        idxb = ep.tile([P, G, E], U32)
        wpairs = ep.tile([P, G, TOPK], F32)
        wout = ep.tile([P, G, TOPK], F32)
        iout = ep.tile([P, G, TOPK], I32)

        # output DMAs deferred one block: their wout/iout semaphore waits
        # must never sit ahead of stream DMAs in a queue
        pending_out = []

        def score_block(b, pt):
            g0, g1 = b * GPB, (b + 1) * GPB
            gs = GPB
            sh = [P, gs, E]

            def f32t(name):
                return sc.tile(sh, F32, tag=name, name=f"{name}_{b % 2}")

            L = pt[:]                     # [P, GPB, E] logits in PSUM
            a = f32t("a")
            nc.vector.tensor_scalar(a[:].bitcast(I32), L.bitcast(I32),
                                    0x7FFFFFFF, None, op0=OP.bitwise_and)
            yn = f32t("yn")
            nc.vector.tensor_scalar_mul(yn, a, NEG_INV_LN2)
            mi = sc.tile(sh, I32, tag="mi", name=f"mi_{b % 2}")
            nc.vector.tensor_copy(mi, yn)                  # f32 -> i32
            mf = f32t("mf")
            nc.vector.tensor_copy(mf, mi)                  # i32 -> f32
            g2 = f32t("g2")
            nc.vector.scalar_tensor_tensor(g2, mf, LN2_HI, a, op0=OP.mult, op1=OP.add)
            nc.vector.scalar_tensor_tensor(g2, mf, LN2_LO, g2, op0=OP.mult, op1=OP.add)
            rt = f32t("rt")
            deg = len(EXP_C) - 1
            nc.vector.tensor_scalar_mul(rt, g2, EXP_C[deg])
            for k in range(deg - 1, 0, -1):
                nc.vector.scalar_tensor_tensor(rt, rt, EXP_C[k], g2, op0=OP.add, op1=OP.mult)
            eb = sc.tile(sh, I32, tag="eb", name=f"eb_{b % 2}")
            nc.vector.tensor_scalar_add(eb, mi, 127)
            nc.vector.tensor_scalar(eb, eb, 23, None, op0=OP.logical_shift_left)
            t = f32t("t")
            nc.vector.scalar_tensor_tensor(t, rt, EXP_C[0], eb[:].bitcast(F32),
                                           op0=OP.add, op1=OP.mult)
            den = f32t("den")
            nc.vector.tensor_scalar_add(den, t, 2.0)
            rd = f32t("rd")
            m0 = f32t("m0")
            nc.vector.reciprocal_approx_accurate(rd, den, scratch=m0)
            z = f32t("z")
            nc.vector.tensor_mul(z, t, rd)
            v = f32t("v")
            nc.vector.tensor_mul(v, z, z)
            ldeg = len(LN_C) - 1
            nc.vector.tensor_scalar_mul(rt, v, LN_C[ldeg])
            for k in range(ldeg - 1, 0, -1):
                nc.vector.scalar_tensor_tensor(rt, rt, LN_C[k], v, op0=OP.add, op1=OP.mult)
            u = f32t("u")
            nc.vector.scalar_tensor_tensor(u, rt, LN_C[0], z, op0=OP.add, op1=OP.mult)
            sp = f32t("sp")
            nc.vector.scalar_tensor_tensor(sp, L, 0.0, u, op0=OP.max, op1=OP.add)
            s0 = f32t("s0")
            nc.scalar.activation(s0, sp, AF.Sqrt)
            rs = f32t("rs")
            m1 = f32t("m1")
            nc.vector.reciprocal_approx_accurate(rs, s0, scratch=m1)
            s = f32t("s")
            nc.vector.scalar_tensor_tensor(s, sp, 0.5, rs, op0=OP.mult, op1=OP.mult)
            nc.vector.scalar_tensor_tensor(s, s0, 0.5, s, op0=OP.mult, op1=OP.add)
            biased = f32t("biased")
            brep_b = brep[:].unsqueeze(1).broadcast_to(sh)
            nc.vector.tensor_add(biased, s, brep_b)

            for g in range(g0, g1):
                gl = g - g0
                nc.vector.max(maxb[:, g, :], biased[:, gl, :])
                nc.vector.max_index(idxb[:, g, :], maxb[:, g, :], biased[:, gl, :])
            # indices are final right after max_index: copy now so the DMA
            # (issued next block) overlaps the weight-extraction chain
            nc.vector.tensor_copy(iout[:, g0:g1, :],
                                  idxb[:, g0:g1, 0:TOPK].bitcast(I32))
            pending_out.append(lambda: nc.scalar.dma_start(
                iout_d[:, g0:g1, :], iout[:, g0:g1, :]))
            sh2 = [P, gs, TOPK, E]
            oh = sc.tile(sh2, F32, tag="oh", name=f"oh_{b % 2}")
            tt = sc.tile(sh2, F32, tag="tt", name=f"tt_{b % 2}")
            bias2 = biased[:].unsqueeze(2).broadcast_to(sh2)
            mx2 = maxb[:, g0:g1, 0:TOPK].unsqueeze(3).broadcast_to(sh2)
            s2 = s[:].unsqueeze(2).broadcast_to(sh2)
            nc.vector.tensor_tensor(oh, bias2, mx2, op=OP.is_equal)
            nc.vector.tensor_mul(tt, oh, s2)
            nc.vector.reduce_max(wpairs[:, g0:g1, :], tt, axis=AX)
            ssum = sc.tile([P, gs], F32, tag="ssum", name=f"ssum_{b % 2}")
            nc.vector.reduce_sum(ssum, wpairs[:, g0:g1, :], axis=AX)
            r0 = sc.tile([P, gs], F32, tag="r0", name=f"r0_{b % 2}")
            # 18-bit reciprocal is plenty for the normalized weights
            # (tolerance 2e-2); the top-2 ordering never sees r0
            nc.vector.reciprocal_approx_fast(r0, ssum)
            r0b = r0[:].unsqueeze(2).broadcast_to([P, gs, TOPK])
            nc.vector.tensor_tensor(wout[:, g0:g1, :], wpairs[:, g0:g1, :], r0b,
                                    op=OP.mult)
            pending_out.append(lambda: nc.scalar.dma_start(
                wout_d[:, g0:g1, :], wout[:, g0:g1, :]))

        for b in range(NB):
            t0, t1 = b * S, (b + 1) * S
            # issue in consumption order (PE walks d ascending): each lo
            # group right after the hi group it unblocks
            xh_g = [None] * (DCH // KHI)
            xl_g = [None] * (DCH // KLO)

            def load_hi(i):
                xht = xhp.tile([P, KHI, S], F16, tag="xh")
                nc.sync.dma_start(xht, xh_d[:, i * KHI:(i + 1) * KHI, t0:t1])
                xh_g[i] = xht

            def load_lo(j):
                xlt = xlp.tile([P, KLO, S], F8, tag="xl")
                nc.scalar.dma_start(xlt, xl_d[:, j * KLO:(j + 1) * KLO, t0:t1])
                xl_g[j] = xlt

            load_hi(0), load_lo(0)
            while deferred:
                deferred.pop(0)()
            # previous block's output DMAs go out behind this block's first
            # stream DMAs: their data is ready, so no queue stall
            while pending_out:
                pending_out.pop(0)()
            load_hi(1)
            load_hi(2), load_lo(1), load_hi(3)

            acc = accs[b % 3]
            for d in range(DCH):
                xh_t = xh_g[d // KHI]
                nc.tensor.matmul(
                    acc[32:32 + 2 * E, :], wpair[:, d, :], xh_t[:, d % KHI, :],
                    start=(d == 0), stop=(d == DCH - 1),
                    tile_position=(0, 32))
                if d % 2 == 0:
                    # fp8 DoubleRow: two k-chunks per instruction at 0.5
                    # cycles/row; ISA requires dst partition 0 and 16B
                    # stationary stride (hence the zero-padded w8 columns)
                    xl_t = xl_g[d // KLO]
                    j = d % KLO
                    nc.tensor.matmul(
                        acc[0:2 * E, :], w8[:, d:d + 2, :],
                        xl_t[:, j:j + 2, :],
                        start=(d == 0), stop=(d == DCH - 2),
                        perf_mode=mybir.MatmulPerfMode.DoubleRow,
                        tile_position=(0, 0))

            # transpose+combine via selection matmul: token-major logits.
            # lsb copy runs on DVE so the scalar queue never waits on acc.
            lsb = lsbp.tile([48, S], F32, tag="lsb", name=f"lsb{b % 2}")
            nc.vector.tensor_copy(lsb, acc[0:48, :])
            pt = ptpool.tile([P, GPB, E], F32, tag="pt", name=f"pt{b % 3}")
            for q in range(GPB):
                nc.tensor.matmul(pt[:, q, :], lsb[:, q * P:(q + 1) * P], sel,
                                 start=True, stop=True)

            score_block(b, pt)

        while pending_out:
            pending_out.pop(0)()

    nc.compile()
    return nc


def _prep_inputs(x, weight, bias):
    import ml_dtypes
    f16 = np.float16
    F8 = ml_dtypes.float8_e4m3fn
    wt = np.ascontiguousarray(weight.T).astype(np.float32)      # [D, E]
    whi = wt.astype(f16)
    wlo = (wt - whi.astype(np.float32)).astype(f16)
    w8 = np.zeros((D, 2 * E), np.float32)
    w8[:, :E] = wt * np.float32(WSCALE)
    w8 = w8.astype(F8)
    # [D, E] -> [P, DCH, E] so chunk d's stationary is wpair[:, d, :]
    def to_chunks(a):
        return np.ascontiguousarray(
            a.reshape(DCH, P, a.shape[-1]).transpose(1, 0, 2))
    wpair = np.concatenate([to_chunks(whi), to_chunks(wlo)], axis=2)
    w8_sb = to_chunks(w8)
    brep = np.ascontiguousarray(np.broadcast_to(bias.astype(np.float32), (P, E)))
    sel = np.zeros((48, E), np.float32)
    for e in range(E):
        sel[e, e] = LO_COMB
        sel[32 + e, e] = 1.0
        sel[40 + e, e] = 1.0

    in_maps = []
    for c in range(NCORES):
        xs = x[c * TPC:(c + 1) * TPC]
        xT = np.ascontiguousarray(xs.T).astype(np.float32)      # [D, TPC]
        xhi = xT.astype(f16)
        xl8 = ((xT - xhi.astype(np.float32)) * np.float32(XSCALE)).astype(F8)
        xh_r = np.ascontiguousarray(
            xhi.reshape(DCH, P, TPC).transpose(1, 0, 2))        # [P, DCH, TPC]
        xl_r = np.ascontiguousarray(
            xl8.reshape(DCH, P, TPC).transpose(1, 0, 2))
        in_maps.append({
            "xh": xh_r, "xl8": xl_r,
            "wpair": wpair, "w8": w8_sb,
            "bias_rep": brep, "sel": sel,
        })
    return in_maps


def kernel(x, weight, bias):
    x = np.asarray(x, dtype=np.float32)
    weight = np.asarray(weight, dtype=np.float32)
    bias = np.asarray(bias, dtype=np.float32)
    assert x.shape == (T_FULL, D) and weight.shape == (E, D) and bias.shape == (E,)

    from concourse.bass_utils import run_bass_kernel_spmd

    if "nc" not in _CACHE:
        _CACHE["nc"] = _build_nc()
    nc = _CACHE["nc"]

    in_maps = _prep_inputs(x, weight, bias)
    res = run_bass_kernel_spmd(nc, in_maps, core_ids=list(range(NCORES)),
                               trace=bool(os.environ.get("BASS_TRACE")))
    _CACHE["last_results"] = res

    weights = np.empty((T_FULL, TOPK), np.float32)
    indices = np.empty((T_FULL, TOPK), np.int32)
    for c in range(NCORES):
        w_c = res.results[c]["w_out"]                 # [P, G, 2], token = g*128+p
        i_c = res.results[c]["i_out"]
        weights[c * TPC:(c + 1) * TPC] = w_c.transpose(1, 0, 2).reshape(TPC, TOPK)
        indices[c * TPC:(c + 1) * TPC] = i_c.transpose(1, 0, 2).reshape(TPC, TOPK)
    if ROUTE_SCALE != 1.0:
        weights *= ROUTE_SCALE
    return weights, indices


# revision 6
# speedup vs baseline: 1.0586x; 1.0055x over previous
"""MoE gate (nn_Gate) Trainium2 kernel, v8.

Computes, for x[32768, 4096] f32, weight[8, 4096] f32, bias[8] f32:
    logits  = x @ weight.T
    scores  = sqrt(softplus(logits))
    indices = top2(scores + bias)
    weights = normalize(scores at indices)
returning (weights[32768, 2] f32, indices[32768, 2] int32).

Strategy (8 NeuronCores, data-parallel over tokens, no collectives):
  * 3 bytes/element streamed per x value instead of 4: xh = fp16(x) plus
    xl8 = fp8e3m4((x - xh) * 2^11). The fp8 lo plane carries ~5 extra
    mantissa bits, giving ~2^-16 relative logit accuracy -- enough to
    reproduce the f32 reference's top-2 ordering except on exact ties
    (empirically 1 token in 32768, same as a full hi/lo fp16 split).
    DMA drops from 64 MiB to 48 MiB per core; the DMA engines (360 GB/s
    shared across queues) are the roofline at ~140 us.
  * One fp16 matmul per (chunk, block) with a packed [whi | wlo] 16-wide
    stationary computes both hi products from a single xh stream; the lo
    product xl8 @ w8 (both fp8e3m4) streams once more at fp8 rate.
  * 512-token blocks; block b's transpose + scoring hides under block
    b+1's streaming. Queue discipline keeps the stream dense: sync (SP)
    carries only xh DMAs, scalar (Activation) only xl DMAs + the sqrt;
    the PSUM->SBUF logit copy runs on DVE, and output DMAs are deferred
    one block so their semaphore waits never stall a stream queue.
  * PSUM rows per block-accumulator: 0:16 hi products, 32:40 lo product
    (rows 16:32 zeroed once); a [40, 8] f32 selection matmul transposes
    to token-major and folds in the 2^-16 lo-plane scale in one shot.
    Scoring reads logits straight from the PSUM transpose tile.
  * Scoring: softplus via range-reduced polynomial exp + ln1p (ACT LUT
    Exp/Ln on this build are only ~1e-5 accurate; polynomials keep the
    biased-score error ~1e-7 so top-2 ordering matches f32), sqrt via
    ACT LUT + one Newton step, top-2 via DVE max8/max_index. The three
    reciprocals use the custom-DVE approx ops (22-bit for the two that
    feed the ordering, 18-bit for the final normalize) instead of
    explicit Newton chains.
"""

import os
from contextlib import ExitStack

import numpy as np

T_FULL = 32768
D = 4096
E = 8
NCORES = 8
TPC = T_FULL // NCORES      # tokens per core
P = 128                     # partitions
DCH = D // P                # 32 contraction chunks
S = 512                     # tokens per block (one PSUM bank)
NB = TPC // S               # 8 blocks
GPB = S // P                # 4 groups of 128 tokens per block
G = TPC // P                # 32 groups per core
KHI = 8                     # d-chunks per xh DMA
KLO = 16                    # d-chunks per xl DMA
TOPK = 2
ROUTE_SCALE = 1.0
XSCALE = 2.0 ** 11          # lo plane pre-scale
WSCALE = 2.0 ** 5           # lo-plane weight pre-scale
LO_COMB = 1.0 / (XSCALE * WSCALE)

# exp(-x) on [-0.36, 0.71] (covers either f32->i32 rounding semantics of
# the range reduction), Chebyshev fit, rel err ~6.5e-8
EXP_C = [
    1.0000000248079328, -0.9999996937194584, 0.4999974889120239,
    -0.1666744435603411, 0.04171073643046914, -0.008316109711575575,
    0.0011778843053845457,
]
# H(v) = ln((1+z)/(1-z))/z, v = z^2 in [0, 1/9], rel err ~1e-10
LN_C = [
    1.9999999998089943, 0.6666667902706496, 0.3999871119480547,
    0.28620208897656446, 0.21398543327861763, 0.2439397667369125,
]
LN2_HI = 0.693359375                     # 12-bit, m*LN2_HI exact in f32
LN2_LO = float(np.log(2.0) - 0.693359375)
NEG_INV_LN2 = -1.4426950408889634

_CACHE = {}


def _build_nc():
    import concourse.bacc as bacc
    import concourse.tile as tile
    import concourse.mybir as mybir

    F32 = mybir.dt.float32
    F16 = mybir.dt.float16
    F8 = mybir.dt.float8e4
    I32 = mybir.dt.int32
    U32 = mybir.dt.uint32
    AF = mybir.ActivationFunctionType
    OP = mybir.AluOpType
    AX = mybir.AxisListType.X

    nc = bacc.Bacc("TRN2", target_bir_lowering=False, debug=False)

    xh_d = nc.dram_tensor("xh", [P, DCH, TPC], F16, kind="ExternalInput").ap()
    xl_d = nc.dram_tensor("xl8", [P, DCH, TPC], F8, kind="ExternalInput").ap()
    wp_d = nc.dram_tensor("wpair", [P, DCH, 2 * E], F16, kind="ExternalInput").ap()
    w8_d = nc.dram_tensor("w8", [P, DCH, 2 * E], F8, kind="ExternalInput").ap()
    br_d = nc.dram_tensor("bias_rep", [P, E], F32, kind="ExternalInput").ap()
    sel_d = nc.dram_tensor("sel", [48, E], F32, kind="ExternalInput").ap()
    wout_d = nc.dram_tensor("w_out", [P, G, TOPK], F32, kind="ExternalOutput").ap()
    iout_d = nc.dram_tensor("i_out", [P, G, TOPK], I32, kind="ExternalOutput").ap()

    with tile.TileContext(nc) as tc, ExitStack() as ctx:
        singles = ctx.enter_context(tc.tile_pool(name="singles", bufs=1))
        xhp = ctx.enter_context(tc.tile_pool(name="xhp", bufs=10))
        xlp = ctx.enter_context(tc.tile_pool(name="xlp", bufs=6))
        pspool = ctx.enter_context(tc.tile_pool(name="ps", bufs=3, space="PSUM"))
        ptpool = ctx.enter_context(tc.tile_pool(name="pt", bufs=2, space="PSUM"))
        lsbp = ctx.enter_context(tc.tile_pool(name="lsbp", bufs=2))
        ep = ctx.enter_context(tc.tile_pool(name="ep", bufs=1))
        sc = ctx.enter_context(tc.tile_pool(name="sc", bufs=2))

        # all small input DMAs are deferred behind block 0's first stream
        # transfers: the x stream starts the moment the DMA engines wake,
        # and the PE has ~50us of slack before it needs the weights
        wpair = singles.tile([P, DCH, 2 * E], F16)
        w8 = singles.tile([P, DCH, 2 * E], F8)
        brep = singles.tile([P, E], F32)
        sel = singles.tile([48, E], F32)
        deferred_sync = [lambda: nc.sync.dma_start(wpair, wp_d)]
        deferred = [lambda: nc.scalar.dma_start(w8, w8_d),
                    lambda: nc.scalar.dma_start(brep, br_d),
                    lambda: nc.scalar.dma_start(sel, sel_d)]

        accs = [pspool.tile([P, S], F32, tag="ps", name=f"acc{i}")
                for i in range(3)]
        # Rows 16:32 sit between the two matmul target groups and are read
        # by the selection matmul (sel rows there are 0); zero them once so
        # 0 * garbage can't produce NaN. Engine writes must start at a
        # 32-aligned partition, so zero 0:32 (0:16 is overwritten by the
        # first start=True matmul).
        for a in accs:
            nc.vector.memset(a[0:32, :], 0.0)

        # persistent outputs + scoring state
        maxb = ep.tile([P, G, E], F32)
        idxb = ep.tile([P, G, E], U32)
        wpairs = ep.tile([P, G, TOPK], F32)
        wout = ep.tile([P, G, TOPK], F32)
        iout = ep.tile([P, G, TOPK], I32)

        # output DMAs deferred one block: their wout/iout semaphore waits
        # must never sit ahead of stream DMAs in a queue
        pending_out = []

        def score_block_ops(b, g0, gs, pt, imm=False):
            """Score chain as a list of op closures, so the caller can emit
            two blocks' chains zip-interleaved (each chain's semaphore
            latency hides under the other's engine time)."""
            g1 = g0 + gs
            sh = [P, gs, E]
            ops = []
            emit = ops.append

            def f32t(name):
                return sc.tile(sh, F32, tag=f"{name}{gs}", name=f"{name}_{b % 2}_{gs}")

            L = pt[:]                     # [P, gs, E] logits in PSUM
            a = f32t("a")
            emit(lambda: nc.vector.tensor_scalar(
                a[:].bitcast(I32), L.bitcast(I32), 0x7FFFFFFF, None,
                op0=OP.bitwise_and))
            yn = f32t("yn")
            emit(lambda: nc.vector.tensor_scalar_mul(yn, a, NEG_INV_LN2))
            mi = sc.tile(sh, I32, tag=f"mi{gs}", name=f"mi_{b % 2}_{gs}")
            emit(lambda: nc.vector.tensor_copy(mi, yn))      # f32 -> i32
            mf = f32t("mf")
            emit(lambda: nc.vector.tensor_copy(mf, mi))      # i32 -> f32
            g2 = f32t("g2")
            emit(lambda: nc.vector.scalar_tensor_tensor(
                g2, mf, LN2_HI, a, op0=OP.mult, op1=OP.add))
            emit(lambda: nc.vector.scalar_tensor_tensor(
                g2, mf, LN2_LO, g2, op0=OP.mult, op1=OP.add))
            rt = f32t("rt")
            deg = len(EXP_C) - 1
            emit(lambda: nc.vector.tensor_scalar_mul(rt, g2, EXP_C[deg]))
            for k in range(deg - 1, 0, -1):
                emit(lambda k=k: nc.vector.scalar_tensor_tensor(
                    rt, rt, EXP_C[k], g2, op0=OP.add, op1=OP.mult))
            eb = sc.tile(sh, I32, tag=f"eb{gs}", name=f"eb_{b % 2}_{gs}")
            emit(lambda: nc.vector.tensor_scalar_add(eb, mi, 127))
            emit(lambda: nc.vector.tensor_scalar(
                eb, eb, 23, None, op0=OP.logical_shift_left))
            t = f32t("t")
            emit(lambda: nc.vector.scalar_tensor_tensor(
                t, rt, EXP_C[0], eb[:].bitcast(F32), op0=OP.add, op1=OP.mult))
            den = f32t("den")
            emit(lambda: nc.vector.tensor_scalar_add(den, t, 2.0))
            rd = f32t("rd")
            m0 = f32t("m0")
            emit(lambda: nc.vector.reciprocal_approx_accurate(rd, den, scratch=m0))
            z = f32t("z")
            emit(lambda: nc.vector.tensor_mul(z, t, rd))
            v = f32t("v")
            emit(lambda: nc.vector.tensor_mul(v, z, z))
            ldeg = len(LN_C) - 1
            emit(lambda: nc.vector.tensor_scalar_mul(rt, v, LN_C[ldeg]))
            for k in range(ldeg - 1, 0, -1):
                emit(lambda k=k: nc.vector.scalar_tensor_tensor(
                    rt, rt, LN_C[k], v, op0=OP.add, op1=OP.mult))
            u = f32t("u")
            emit(lambda: nc.vector.scalar_tensor_tensor(
                u, rt, LN_C[0], z, op0=OP.add, op1=OP.mult))
            sp = f32t("sp")
            emit(lambda: nc.vector.scalar_tensor_tensor(
                sp, L, 0.0, u, op0=OP.max, op1=OP.add))
            s0 = f32t("s0")
            emit(lambda: nc.scalar.activation(s0, sp, AF.Sqrt))
            rs = f32t("rs")
            m1 = f32t("m1")
            emit(lambda: nc.vector.reciprocal_approx_accurate(rs, s0, scratch=m1))
            s = f32t("s")
            emit(lambda: nc.vector.scalar_tensor_tensor(
                s, sp, 0.5, rs, op0=OP.mult, op1=OP.mult))
            emit(lambda: nc.vector.scalar_tensor_tensor(
                s, s0, 0.5, s, op0=OP.mult, op1=OP.add))
            biased = f32t("biased")
            brep_b = brep[:].unsqueeze(1).broadcast_to(sh)
            emit(lambda: nc.vector.tensor_add(biased, s, brep_b))

            for g in range(g0, g1):
                gl = g - g0
                emit(lambda g=g, gl=gl: nc.vector.max(
                    maxb[:, g, :], biased[:, gl, :]))
                emit(lambda g=g, gl=gl: nc.vector.max_index(
                    idxb[:, g, :], maxb[:, g, :], biased[:, gl, :]))
            # indices are final right after max_index: copy now so the DMA
            # (issued next block) overlaps the weight-extraction chain
            emit(lambda: nc.vector.tensor_copy(
                iout[:, g0:g1, :], idxb[:, g0:g1, 0:TOPK].bitcast(I32)))
            if imm:
                emit(lambda: nc.scalar.dma_start(
                    iout_d[:, g0:g1, :], iout[:, g0:g1, :]))
            else:
                emit(lambda: pending_out.append(lambda: nc.scalar.dma_start(
                    iout_d[:, g0:g1, :], iout[:, g0:g1, :])))
            sh2 = [P, gs, TOPK, E]
            oh = sc.tile(sh2, F32, tag=f"oh{gs}", name=f"oh_{b % 2}_{gs}")
            tt = sc.tile(sh2, F32, tag=f"tt{gs}", name=f"tt_{b % 2}_{gs}")
            bias2 = biased[:].unsqueeze(2).broadcast_to(sh2)
            mx2 = maxb[:, g0:g1, 0:TOPK].unsqueeze(3).broadcast_to(sh2)
            s2 = s[:].unsqueeze(2).broadcast_to(sh2)
            emit(lambda: nc.vector.tensor_tensor(oh, bias2, mx2, op=OP.is_equal))
            emit(lambda: nc.vector.tensor_mul(tt, oh, s2))
            emit(lambda: nc.vector.reduce_max(wpairs[:, g0:g1, :], tt, axis=AX))
            ssum = sc.tile([P, gs], F32, tag=f"ssum{gs}", name=f"ssum_{b % 2}_{gs}")
            emit(lambda: nc.vector.reduce_sum(ssum, wpairs[:, g0:g1, :], axis=AX))
            r0 = sc.tile([P, gs], F32, tag=f"r0{gs}", name=f"r0_{b % 2}_{gs}")
            # 18-bit reciprocal is plenty for the normalized weights
            # (tolerance 2e-2); the top-2 ordering never sees r0
            emit(lambda: nc.vector.reciprocal_approx_fast(r0, ssum))
            r0b = r0[:].unsqueeze(2).broadcast_to([P, gs, TOPK])
            emit(lambda: nc.vector.tensor_tensor(
                wout[:, g0:g1, :], wpairs[:, g0:g1, :], r0b, op=OP.mult))
            if imm:
                emit(lambda: nc.scalar.dma_start(
                    wout_d[:, g0:g1, :], wout[:, g0:g1, :]))
            else:
                emit(lambda: pending_out.append(lambda: nc.scalar.dma_start(
                    wout_d[:, g0:g1, :], wout[:, g0:g1, :])))
            return ops

        blocks = [(i * S, S) for i in range(NB - 1)]
        blocks += [((NB - 1) * S, S // 2), ((NB - 1) * S + S // 2, S // 2)]

        prev_lo = None
        for b, (t0, Sb) in enumerate(blocks):
            t1 = t0 + Sb
            gs = Sb // P
            xh_g = [None] * (DCH // KHI)
            xl_g = [None] * (DCH // KLO)
            lo_t0 = (t0 // S) * S            # lo superblock origin
            lo_off = t0 - lo_t0

            def load_hi(i, t0=t0, t1=t1, Sb=Sb, xh_g=xh_g):
                xht = xhp.tile([P, KHI, Sb], F16, tag=f"xh{Sb}")
                nc.sync.dma_start(xht, xh_d[:, i * KHI:(i + 1) * KHI, t0:t1])
                xh_g[i] = xht

            def load_lo(j, lo_t0=lo_t0, xl_g=xl_g):
                xlt = xlp.tile([P, KLO, S], F8, tag="xl")
                nc.scalar.dma_start(
                    xlt, xl_d[:, j * KLO:(j + 1) * KLO, lo_t0:lo_t0 + S])
                xl_g[j] = xlt

            new_lo = lo_off == 0
            if not new_lo:
                xl_g[:] = prev_lo
            # issue in consumption order (PE walks d ascending): each lo
            # group right after the hi group it unblocks
            load_hi(0)
            if new_lo:
                load_lo(0)
            while deferred_sync:
                deferred_sync.pop(0)()
            while deferred:
                deferred.pop(0)()
            # previous block's output DMAs go out behind this block's first
            # stream DMAs: their data is ready, so no queue stall
            while pending_out:
                pending_out.pop(0)()
            load_hi(1)
            load_hi(2)
            if new_lo:
                load_lo(1)
            load_hi(3)
            prev_lo = list(xl_g)

            acc = accs[b % 3]
            for d in range(DCH):
                nc.tensor.matmul(
                    acc[32:32 + 2 * E, :Sb], wpair[:, d, :],
                    xh_g[d // KHI][:, d % KHI, :],
                    start=(d == 0), stop=(d == DCH - 1),
                    tile_position=(0, 32))
                if d % 2 == 0:
                    # fp8 DoubleRow: two k-chunks per instruction at 0.5
                    # cycles/row; ISA requires dst partition 0 and 16B
                    # stationary stride (hence the zero-padded w8 columns)
                    xl_t = xl_g[d // KLO]
                    j = d % KLO
                    nc.tensor.matmul(
                        acc[0:2 * E, :Sb], w8[:, d:d + 2, :],
                        xl_t[:, j:j + 2, lo_off:lo_off + Sb],
                        start=(d == 0), stop=(d == DCH - 2),
                        perf_mode=mybir.MatmulPerfMode.DoubleRow,
                        tile_position=(0, 0))

            # transpose+combine via selection matmul: token-major logits.
            # lsb copy runs on DVE so the scalar queue never waits on acc.
            lsb = lsbp.tile([48, Sb], F32, tag=f"lsb{Sb}", name=f"lsb{b % 2}_{Sb}")
            nc.vector.tensor_copy(lsb, acc[0:48, :Sb])
            pt = ptpool.tile([P, gs, E], F32, tag=f"pt{Sb}", name=f"pt{b % 3}_{Sb}")
            for q in range(gs):
                nc.tensor.matmul(pt[:, q, :], lsb[:, q * P:(q + 1) * P], sel,
                                 start=True, stop=True)

            ops = score_block_ops(b, t0 // P, gs, pt,
                                  imm=b >= len(blocks) - 2)
            if b < len(blocks) - 2:
                for f in ops:
                    f()
            elif b == len(blocks) - 2:
                held_ops = ops
            else:
                # interleave the last two blocks' chains (staggered so the
                # second chain's data has arrived by the time its first ops
                # dispatch): each chain's per-op semaphore latency hides
                # under the other's engine time
                STAG = 10
                for f in held_ops[:STAG]:
                    f()
                tail1 = held_ops[STAG:]
                for i in range(max(len(tail1), len(ops))):
                    if i < len(tail1):
                        tail1[i]()
                    if i < len(ops):
                        ops[i]()

        while pending_out:
            pending_out.pop(0)()

    nc.compile()
    return nc


def _prep_inputs(x, weight, bias):
    import ml_dtypes
    f16 = np.float16
    F8 = ml_dtypes.float8_e4m3fn
    wt = np.ascontiguousarray(weight.T).astype(np.float32)      # [D, E]
    whi = wt.astype(f16)
    wlo = (wt - whi.astype(np.float32)).astype(f16)
    w8 = np.zeros((D, 2 * E), np.float32)
    w8[:, :E] = wt * np.float32(WSCALE)
    w8 = w8.astype(F8)
    # [D, E] -> [P, DCH, E] so chunk d's stationary is wpair[:, d, :]
    def to_chunks(a):
        return np.ascontiguousarray(
            a.reshape(DCH, P, a.shape[-1]).transpose(1, 0, 2))
    wpair = np.concatenate([to_chunks(whi), to_chunks(wlo)], axis=2)
    w8_sb = to_chunks(w8)
    brep = np.ascontiguousarray(np.broadcast_to(bias.astype(np.float32), (P, E)))
    sel = np.zeros((48, E), np.float32)
    for e in range(E):
        sel[e, e] = LO_COMB
        sel[32 + e, e] = 1.0
        sel[40 + e, e] = 1.0

    in_maps = []
    for c in range(NCORES):
        xs = x[c * TPC:(c + 1) * TPC]
        xT = np.ascontiguousarray(xs.T).astype(np.float32)      # [D, TPC]
        xhi = xT.astype(f16)
        xl8 = ((xT - xhi.astype(np.float32)) * np.float32(XSCALE)).astype(F8)
        xh_r = np.ascontiguousarray(
            xhi.reshape(DCH, P, TPC).transpose(1, 0, 2))        # [P, DCH, TPC]
        xl_r = np.ascontiguousarray(
            xl8.reshape(DCH, P, TPC).transpose(1, 0, 2))
        in_maps.append({
            "xh": xh_r, "xl8": xl_r,
            "wpair": wpair, "w8": w8_sb,
            "bias_rep": brep, "sel": sel,
        })
    return in_maps


def kernel(x, weight, bias):
    x = np.asarray(x, dtype=np.float32)
    weight = np.asarray(weight, dtype=np.float32)
    bias = np.asarray(bias, dtype=np.float32)
    assert x.shape == (T_FULL, D) and weight.shape == (E, D) and bias.shape == (E,)

    from concourse.bass_utils import run_bass_kernel_spmd

    if "nc" not in _CACHE:
        _CACHE["nc"] = _build_nc()
    nc = _CACHE["nc"]

    in_maps = _prep_inputs(x, weight, bias)
    res = run_bass_kernel_spmd(nc, in_maps, core_ids=list(range(NCORES)),
                               trace=bool(os.environ.get("BASS_TRACE")))
    _CACHE["last_results"] = res

    weights = np.empty((T_FULL, TOPK), np.float32)
    indices = np.empty((T_FULL, TOPK), np.int32)
    for c in range(NCORES):
        w_c = res.results[c]["w_out"]                 # [P, G, 2], token = g*128+p
        i_c = res.results[c]["i_out"]
        weights[c * TPC:(c + 1) * TPC] = w_c.transpose(1, 0, 2).reshape(TPC, TOPK)
        indices[c * TPC:(c + 1) * TPC] = i_c.transpose(1, 0, 2).reshape(TPC, TOPK)
    if ROUTE_SCALE != 1.0:
        weights *= ROUTE_SCALE
    return weights, indices
